# revision 39
# baseline (speedup 1.0000x reference)
"""Trainium2 Bass kernel for nn_Bottleneck (TBN-style quantized bottleneck).

Reference computation (per reference.py):
    identity = x
    h = qconv(BN(x,g1,b1),  w1b, 1x1)          # ternary acts, binary weights
    h = qconv(BN(h,g2,b2),  w2b, 3x3 pad 1)
    h = qconv(BN(h,g3,b3),  w3b, 1x1)
    out = identity + h
where BN uses batch statistics over (N,H,W) (sync-BN across the batch),
ternarize(x) = (x>d) - (x<-d) with d = 0.7*mean|x| (global), and
binarize(w) = sign(w)*mean|w|_per_out_channel.

Sharding: data-parallel over batch, 8 images per core on 8 cores; BN stats
and ternary thresholds are synchronized with one small AllReduce per stats
barrier (4 total: layer-1 needs two, exact Sum|x-m|).

v3 design notes.  In this runtime the dominant execution cost is
per-instruction processing (~0.5 us/instruction), so the kernel is shaped
to MINIMIZE INSTRUCTION COUNT first and engine balance second:
  * All convs are fp8 DoubleRow matmuls.  conv1/conv3 stack the two
    ternary compare-parts (s-0.5 in {-0.5,+0.5}) as the two DR k-tiles
    against duplicated weights: W.(s1-0.5)+W.(s2-0.5) = W.t with no
    explicit ternarize add and no extra PE cost.  DVE images emit fp8
    half-form compares; ACT images use Sign (+-1, evac scale 0.5).
  * conv2 materializes t = s1h+s2h in {-1,0,1} fp8 into a 3-slab padded
    tile [B|A|C] where B is t shifted one column left and C one row up.
    A DR matmul over slabs (B,A) with weights [w_{t+1}, w_t] computes two
    horizontal taps at once; slabs (A,C) pair taps (2,5); tap 8 rides a
    DR with a zero weight slab.  5 matmuls per half-image instead of 18.
  * One activation-table set (Copy/Square/Abs/Sign/Abs_reciprocal_sqrt),
    rsqrt = table seed + 1 mult-only Newton step; 1/A = u*r*(1/g)*(1/a).
  * Outputs are stored per (img, q-pair) with a DRAM layout that matches
    SBUF ([IMGS, 2, 128, 2, HW]), halving store DMAs; loads are 2-image;
    big DMAs alternate the sync (HWDGE) and gpsimd (SWDGE) queues.
  * BN + next-layer ternarize folded into per-channel thresholds a1, a2 on
    raw integer conv outputs (z kept bf16-exact).
"""

import os
from contextlib import ExitStack

import numpy as np
import ml_dtypes

import concourse.bass as bass
import concourse.bacc as bacc
import concourse.tile as tile
import concourse.mybir as mybir
from concourse import bass_isa
from concourse.bass_utils import run_bass_kernel_spmd

F32 = mybir.dt.float32
BF16 = mybir.dt.bfloat16
FP8 = mybir.dt.float8e4
AF = mybir.ActivationFunctionType
OP = mybir.AluOpType
PM = mybir.MatmulPerfMode

N_CORES = 8
IMGS = 8          # images per core
HW = 784          # 28*28
H = 28
EPS = 1e-5
N1 = 64 * HW              # BN count per channel (global batch)
NTOT1 = 64 * 512 * HW     # element count for delta1
NTOT2 = 64 * 128 * HW     # element count for delta2/delta3

_CACHE = {}

# per-image engine for layer-1 ternarize compares:
#   'v' = DVE fp8 half-form, 'a' = ACT Sign fp8 (evac scale 0.5)
L1_ENG = ['v', 'a', 'v', 'a', 'v', 'a', 'v', 'a']
L3_ENG = ['v', 'v', 'a', 'a', 'v', 'v', 'a', 'a']


def _rsqrt(nc, pool, u, shape, tag):
    """r = 1/sqrt(u), u > 0: ACT table seed (~4e-5 rel) + 1 mult-only
    Newton step on DVE (-> ~3e-9, ~ulp)."""
    r = pool.tile(shape, F32, tag=f"{tag}_r0", name=f"{tag}_r0")
    nc.scalar.activation(out=r[:], in_=u[:], func=AF.Abs_reciprocal_sqrt)
    for i in range(1):
        w1 = pool.tile(shape, F32, tag=f"{tag}_w1_{i}", name=f"{tag}_w1_{i}")
        nc.vector.tensor_mul(w1[:], u[:], r[:])
        w2 = pool.tile(shape, F32, tag=f"{tag}_w2_{i}", name=f"{tag}_w2_{i}")
        nc.vector.tensor_mul(w2[:], w1[:], r[:])
        h = pool.tile(shape, F32, tag=f"{tag}_h_{i}", name=f"{tag}_h_{i}")
        nc.vector.tensor_scalar(out=h[:], in0=w2[:], scalar1=-0.5, scalar2=1.5,
                                op0=OP.mult, op1=OP.add)
        r2 = pool.tile(shape, F32, tag=f"{tag}_r_{i}", name=f"{tag}_r_{i}")
        nc.vector.tensor_mul(r2[:], r[:], h[:])
        r = r2
    return r


def _stats_stage1(nc, pool, tag, nchunk, sx, sq, gv, alpha, n_cnt,
                  ginv, alphainv=None):
    """Mean / rstd / slope from AllReduced Sum z, Sum z^2."""
    shape = [128, nchunk]

    def t(name):
        return pool.tile(shape, F32, tag=f"{tag}_{name}", name=f"{tag}_{name}")

    m = t("m")
    nc.vector.tensor_scalar(out=m[:], in0=sx[:], scalar1=1.0 / n_cnt,
                            scalar2=None, op0=OP.mult)
    negm = t("negm")
    nc.vector.tensor_scalar(out=negm[:], in0=m[:], scalar1=-1.0, scalar2=None,
                            op0=OP.mult)
    ex2 = t("ex2")
    nc.vector.tensor_scalar(out=ex2[:], in0=sq[:], scalar1=1.0 / n_cnt,
                            scalar2=None, op0=OP.mult)
    m2 = t("m2")
    nc.vector.tensor_mul(m2[:], m[:], m[:])
    v = t("v")
    nc.vector.tensor_sub(v[:], ex2[:], m2[:])
    if alpha is not None:
        asq = t("asq")
        nc.vector.tensor_mul(asq[:], alpha[:], alpha[:])
        vh = t("vh")
        nc.vector.tensor_mul(vh[:], v[:], asq[:])
    else:
        vh = v
    u = t("u")
    nc.vector.tensor_scalar(out=u[:], in0=vh[:], scalar1=EPS, scalar2=None,
                            op0=OP.add)
    r = _rsqrt(nc, pool, u, shape, f"{tag}_rs")
    A = t("A")
    nc.vector.tensor_mul(A[:], r[:], gv[:])
    if alpha is not None:
        A2 = t("A2")
        nc.vector.tensor_mul(A2[:], A[:], alpha[:])
        A = A2
    # 1/A = sqrt(u) * (1/g) * (1/alpha); sqrt(u) = u * r
    sq_u = t("squ")
    nc.vector.tensor_mul(sq_u[:], u[:], r[:])
    Ainv = t("Ainv")
    nc.vector.tensor_mul(Ainv[:], sq_u[:], ginv[:])
    if alphainv is not None:
        A3 = t("Ainv2")
        nc.vector.tensor_mul(A3[:], Ainv[:], alphainv[:])
        Ainv = A3
    return {"m": m, "negm": negm, "A": A, "Ainv": Ainv, "shape": shape}


def _stats_stage2(nc, pool, tag, st, sa, bv, n_tot, want_neg=True):
    """Thresholds from stage-1 stats + AllReduced Sum|z - m| (or Sum|z|)."""
    shape = st["shape"]
    m, A, Ainv = st["m"], st["A"], st["Ainv"]
    nchunk = shape[1]

    def t(name):
        return pool.tile(shape, F32, tag=f"{tag}_{name}", name=f"{tag}_{name}")

    say = t("say")
    nc.vector.tensor_mul(say[:], A[:], sa[:])
    srow = pool.tile([128, 1], F32, tag=f"{tag}_srow", name=f"{tag}_srow")
    if nchunk > 1:
        nc.vector.tensor_reduce(out=srow[:], in_=say[:],
                                axis=mybir.AxisListType.X, op=OP.add)
    else:
        nc.vector.tensor_copy(srow[:], say[:])
    sall = pool.tile([128, 1], F32, tag=f"{tag}_sall", name=f"{tag}_sall")
    nc.gpsimd.partition_all_reduce(sall[:], srow[:], 128, bass_isa.ReduceOp.add)
    delta = pool.tile([128, 1], F32, tag=f"{tag}_delta", name=f"{tag}_delta")
    nc.vector.tensor_scalar(out=delta[:], in0=sall[:], scalar1=0.7 / n_tot,
                            scalar2=None, op0=OP.mult)
    d1 = t("d1")
    nc.vector.tensor_scalar(out=d1[:], in0=bv[:], scalar1=delta[:], scalar2=-1.0,
                            op0=OP.subtract, op1=OP.mult)
    e1 = t("e1")
    nc.vector.tensor_mul(e1[:], d1[:], Ainv[:])
    a1 = t("a1")
    nc.vector.tensor_add(a1[:], e1[:], m[:])
    d2 = t("d2")
    nc.vector.tensor_scalar(out=d2[:], in0=bv[:], scalar1=delta[:], scalar2=-1.0,
                            op0=OP.add, op1=OP.mult)
    e2 = t("e2")
    nc.vector.tensor_mul(e2[:], d2[:], Ainv[:])
    a2 = t("a2")
    nc.vector.tensor_add(a2[:], e2[:], m[:])
    if not want_neg:
        return a1, a2, None, None
    na1 = t("na1")
    nc.vector.tensor_scalar(out=na1[:], in0=a1[:], scalar1=-1.0, scalar2=None,
                            op0=OP.mult)
    na2 = t("na2")
    nc.vector.tensor_scalar(out=na2[:], in0=a2[:], scalar1=-1.0, scalar2=None,
                            op0=OP.mult)
    return a1, a2, na1, na2


def _emit(ctx: ExitStack, tc: tile.TileContext, x_d, w1_d, w2_d, w3_d,
          cst_d, out_d, single_core=False, repeats=1, no_collective=False):
    nc = tc.nc

    def allreduce(ins, outs):
        if single_core or no_collective:
            nc.gpsimd.dma_start(out=outs[0], in_=ins[0])
        else:
            nc.gpsimd.collective_compute(
                "AllReduce", OP.add, replica_groups=[list(range(N_CORES))],
                ins=ins, outs=outs)

    xpool = ctx.enter_context(tc.tile_pool(name="xres", bufs=1))
    zpool = ctx.enter_context(tc.tile_pool(name="zres", bufs=1))
    wpool = ctx.enter_context(tc.tile_pool(name="wts", bufs=1))
    stpool = ctx.enter_context(tc.tile_pool(name="stats", bufs=1))
    tiny = ctx.enter_context(tc.tile_pool(name="tiny", bufs=1))
    spool = ctx.enter_context(tc.tile_pool(name="scratch", bufs=2))
    padp = ctx.enter_context(tc.tile_pool(name="pads", bufs=1))
    opool = ctx.enter_context(tc.tile_pool(name="outbuf", bufs=2))
    psum = ctx.enter_context(tc.tile_pool(name="psum", bufs=4, space="PSUM"))
    dram = ctx.enter_context(tc.tile_pool(name="dram", bufs=1, space="DRAM"))

    # ---- resident tensors ----
    xt = xpool.tile([128, 4, IMGS, HW], F32, tag="x", name="x")
    z1 = zpool.tile([128, IMGS, HW], BF16, tag="z1", name="z1")
    z2 = zpool.tile([128, IMGS, HW], BF16, tag="z2", name="z2")
    w1s = wpool.tile([128, 8, 128], FP8, tag="w1", name="w1")     # q dup
    w2s = wpool.tile([128, 10, 128], FP8, tag="w2", name="w2")    # tap pairs
    w3s = wpool.tile([128, 8, 128], FP8, tag="w3", name="w3")     # q dup
    csts = wpool.tile([128, 30], F32, tag="cst", name="cst")

    nc.sync.dma_start(out=w1s[:], in_=w1_d[:].rearrange("q k m -> k q m"))
    nc.sync.dma_start(out=w2s[:], in_=w2_d[:].rearrange("q k m -> k q m"))
    nc.sync.dma_start(out=w3s[:], in_=w3_d[:].rearrange("q k m -> k q m"))
    nc.sync.dma_start(out=csts[:], in_=cst_d[:])
    g1c = csts[:, 0:4]
    b1c = csts[:, 4:8]
    al1 = csts[:, 8:9]
    g2c = csts[:, 9:10]
    b2c = csts[:, 10:11]
    al2 = csts[:, 11:12]
    g3c = csts[:, 12:13]
    b3c = csts[:, 13:14]
    al3 = csts[:, 14:18]
    g1i = csts[:, 18:22]
    al1i = csts[:, 22:23]
    g2i = csts[:, 23:24]
    al2i = csts[:, 24:25]
    g3i = csts[:, 25:26]
    al3h = csts[:, 26:30]      # 0.5 * al3 (Sign-form conv3 images)

    # ---- stats accumulators ----
    st1x = stpool.tile([128, 8], F32, tag="st1x", name="st1x")    # sum x
    st1q = stpool.tile([128, 8], F32, tag="st1q", name="st1q")    # sum x^2
    st1a = stpool.tile([128, 32], F32, tag="st1a", name="st1a")   # sum |x-m|
    stz = {}
    for L in (2, 3):
        for k in ("x", "q", "a"):
            n = IMGS if k == "x" else 4
            stz[(L, k)] = stpool.tile([128, n], F32, tag=f"st{L}{k}",
                                      name=f"st{L}{k}")

    # conv2 padded tiles [B|A|C]: 3 rotating, fully zeroed once per rep
    pads = [padp.tile([128, 3, 30, 32], FP8, tag=f"pad{i}", name=f"pad{i}")
            for i in range(3)]

    for _rep in range(repeats):
        for p in pads:
            nc.gpsimd.memset(p[:], 0.0)

        # ================= P1: load x + layer-1 stats =================
        # 2-image loads alternating HWDGE/SWDGE; per half-batch (4 imgs):
        # Sum x (DVE tensor_reduce) and Sum x^2 (ACT Square accum) per q.
        for pr in range(4):
            for sub in range(2):
                img = 2 * pr + sub
                ld_eng = nc.sync if img % 2 == 0 else nc.gpsimd
                ld_eng.dma_start(out=xt[:, :, img, :],
                                 in_=x_d[img].rearrange("q p s -> p q s"))
            if pr % 2 == 1:
                half = pr // 2
                for q in range(4):
                    xs = xt[:, q, 4 * half:4 * half + 4, :]
                    nc.vector.tensor_reduce(
                        out=st1x[:, q * 2 + half:q * 2 + half + 1],
                        in_=xs.rearrange("p a b -> p (a b)"),
                        axis=mybir.AxisListType.X, op=OP.add)
                    dw = spool.tile([128, 4, HW], BF16, tag="sqdump",
                                    name="sqdump", bufs=1)
                    nc.scalar.activation(
                        out=dw[:], in_=xs, func=AF.Square,
                        accum_out=st1q[:, q * 2 + half:q * 2 + half + 1])

        pk1 = stpool.tile([128, 8], F32, tag="pk1", name="pk1")
        for q in range(4):
            nc.vector.tensor_reduce(
                out=pk1[:, q:q + 1], in_=st1x[:, q * 2:q * 2 + 2],
                axis=mybir.AxisListType.X, op=OP.add)
            nc.vector.tensor_reduce(
                out=pk1[:, 4 + q:5 + q], in_=st1q[:, q * 2:q * 2 + 2],
                axis=mybir.AxisListType.X, op=OP.add)
        ar1i = dram.tile([128, 8], F32, tag="ar1i", name="ar1i")
        ar1o = dram.tile([128, 8], F32, tag="ar1o", name="ar1o",
                         addr_space="Shared")
        nc.sync.dma_start(out=ar1i[:], in_=pk1[:])
        allreduce([ar1i.opt()], [ar1o.opt()])
        gp1 = stpool.tile([128, 8], F32, tag="gp1", name="gp1")
        nc.sync.dma_start(out=gp1[:], in_=ar1o[:])

        st1 = _stats_stage1(nc, tiny, "th1", 4, gp1[:, 0:4], gp1[:, 4:8],
                            g1c, None, N1, ginv=g1i)

        # ============ P2: exact Sum|x - m| pass ============
        # ACT 1-pass (Abs, bias=-m) imgs 0-4; DVE pair (5,6); Pool img 7
        # (sub) + DVE abs-reduce.
        for q in range(4):
            for g2 in range(2):
                dw3 = spool.tile([128, 2, HW], BF16, tag="absdump",
                                 name="absdump", bufs=1)
                nc.scalar.activation(out=dw3[:], in_=xt[:, q, 2 * g2:2 * g2 + 2, :],
                                     func=AF.Abs, bias=st1["negm"][:, q:q + 1],
                                     scale=1.0,
                                     accum_out=st1a[:, q * 8 + g2:q * 8 + g2 + 1])
            dw1 = spool.tile([128, HW], BF16, tag="d784", name="absdump1",
                             bufs=2)
            nc.scalar.activation(out=dw1[:], in_=xt[:, q, 4, :],
                                 func=AF.Abs, bias=st1["negm"][:, q:q + 1],
                                 scale=1.0,
                                 accum_out=st1a[:, q * 8 + 2:q * 8 + 3])
            dfp = spool.tile([128, 2, HW], F32, tag="dfp2", name="dfp2", bufs=1)
            nc.vector.tensor_scalar(out=dfp[:], in0=xt[:, q, 5:7, :],
                                    scalar1=st1["m"][:, q:q + 1], scalar2=None,
                                    op0=OP.subtract)
            nc.vector.tensor_reduce(
                out=st1a[:, q * 8 + 3:q * 8 + 4],
                in_=dfp[:].rearrange("p a b -> p (a b)"),
                axis=mybir.AxisListType.X, op=OP.add,
                apply_absolute_value=True)
            dfp1 = spool.tile([128, HW], F32, tag="dfp1", name="dfp1", bufs=1)
            nc.gpsimd.tensor_scalar(out=dfp1[:], in0=xt[:, q, 7, :],
                                    scalar1=st1["m"][:, q:q + 1], scalar2=None,
                                    op0=OP.subtract)
            nc.vector.tensor_reduce(
                out=st1a[:, q * 8 + 4:q * 8 + 5],
                in_=dfp1[:], axis=mybir.AxisListType.X, op=OP.add,
                apply_absolute_value=True)
        pka = stpool.tile([128, 4], F32, tag="pka", name="pka")
        for q in range(4):
            nc.vector.tensor_reduce(out=pka[:, q:q + 1],
                                    in_=st1a[:, q * 8:q * 8 + 5],
                                    axis=mybir.AxisListType.X, op=OP.add)
        arai = dram.tile([128, 4], F32, tag="arai", name="arai")
        arao = dram.tile([128, 4], F32, tag="arao", name="arao",
                         addr_space="Shared")
        nc.sync.dma_start(out=arai[:], in_=pka[:])
        allreduce([arai.opt()], [arao.opt()])
        gpa = stpool.tile([128, 4], F32, tag="gpa", name="gpa")
        nc.sync.dma_start(out=gpa[:], in_=arao[:])

        a1_1, a2_1, na1_1, na2_1 = _stats_stage2(nc, tiny, "th1", st1, gpa[:],
                                                 b1c, NTOT1)

        # ============ P3: ternarize L1 + conv1 (fp8 DR) + L2 stats ============
        p3_tiles = {}

        def p3_cmp(img):
            eng = L1_ENG[img]
            if eng == 'v':
                s8 = spool.tile([128, 4, 2, HW], FP8, tag="s8v", name="s8v",
                                bufs=1)
                for q in range(4):
                    nc.vector.tensor_scalar(
                        out=s8[:, q, 0, :], in0=xt[:, q, img, :],
                        scalar1=a1_1[:, q:q + 1], scalar2=0.5,
                        op0=OP.is_gt, op1=OP.subtract)
                    nc.vector.tensor_scalar(
                        out=s8[:, q, 1, :], in0=xt[:, q, img, :],
                        scalar1=a2_1[:, q:q + 1], scalar2=0.5,
                        op0=OP.is_ge, op1=OP.subtract)
            else:
                s8 = spool.tile([128, 4, 2, HW], FP8, tag="s8a", name="s8a",
                                bufs=1)
                for q in range(4):
                    nc.scalar.activation(out=s8[:, q, 0, :],
                                         in_=xt[:, q, img, :], func=AF.Sign,
                                         bias=na1_1[:, q:q + 1], scale=1.0)
                    nc.scalar.activation(out=s8[:, q, 1, :],
                                         in_=xt[:, q, img, :], func=AF.Sign,
                                         bias=na2_1[:, q:q + 1], scale=1.0)
            p3_tiles[img] = s8

        def p3_conv(img):
            s8 = p3_tiles.pop(img)
            evac_scale = 0.5 if L1_ENG[img] == 'a' else 1.0
            zp = psum.tile([128, 2, 512], F32, tag="zp", name="zp", bufs=4)
            for hh in range(2):
                for q in range(4):
                    nc.tensor.matmul(
                        zp[:, hh, 0:392],
                        w1s[:, 2 * q:2 * q + 2, :],
                        s8[:, q, :, hh * 392:(hh + 1) * 392],
                        start=(q == 0), stop=(q == 3),
                        perf_mode=PM.DoubleRow)
            nc.scalar.activation(
                out=z1[:, img, :].rearrange("p (h s) -> p h s", h=2),
                in_=zp[:, :, 0:392], func=AF.Copy, scale=evac_scale,
                accum_out=stz[(2, "x")][:, img:img + 1])
            if img % 2 == 1:
                pr = img // 2
                zs = z1[:, img - 1:img + 1, :]
                d2t = spool.tile([128, 2, HW], BF16, tag="zsq", name="zsq",
                                 bufs=1)
                nc.vector.scalar_tensor_tensor(
                    out=d2t[:], in0=zs, scalar=1.0, in1=zs,
                    op0=OP.mult, op1=OP.mult,
                    accum_out=stz[(2, "q")][:, pr:pr + 1])
                dat = spool.tile([128, 2, HW], BF16, tag="zab", name="zab",
                                 bufs=1)
                nc.scalar.activation(
                    out=dat[:], in_=zs, func=AF.Abs,
                    accum_out=stz[(2, "a")][:, pr:pr + 1])

        for img in range(IMGS + 1):
            if img < IMGS:
                p3_cmp(img)
            if img >= 1:
                p3_conv(img - 1)

        pk2 = stpool.tile([128, 3], F32, tag="pk2", name="pk2")
        for i, k in enumerate(("x", "q", "a")):
            nc.vector.tensor_reduce(out=pk2[:, i:i + 1], in_=stz[(2, k)][:],
                                    axis=mybir.AxisListType.X, op=OP.add)
        ar2i = dram.tile([128, 3], F32, tag="ar2i", name="ar2i")
        ar2o = dram.tile([128, 3], F32, tag="ar2o", name="ar2o",
                         addr_space="Shared")
        nc.sync.dma_start(out=ar2i[:], in_=pk2[:])
        allreduce([ar2i.opt()], [ar2o.opt()])
        gp2 = stpool.tile([128, 3], F32, tag="gp2", name="gp2")
        nc.sync.dma_start(out=gp2[:], in_=ar2o[:])

        st2 = _stats_stage1(nc, tiny, "th2", 1, gp2[:, 0:1], gp2[:, 1:2],
                            g2c, al1, N1, ginv=g2i, alphainv=al1i)
        a1_2, a2_2, _, _ = _stats_stage2(nc, tiny, "th2", st2, gp2[:, 2:3],
                                         b2c, NTOT2, want_neg=False)

        # ============ P4: ternarize L2 -> 3-slab padded fp8, conv2 ============
        # slabs [B|A|C]: A = t at interior [1:29, 2:30]; B = t at [1:29,
        # 1:29] (column -1); C = t at [0:28, 2:30] (row -1).  DR pairs:
        # (B,A) with weights [w_{t+1}, w_t] for t in {0,3,6}; (A,C) with
        # [w2, w5]; (A,C) with [w8, 0].
        def p4_cmp(img):
            pt = pads[img % 3]
            zi = z1[:, img, :]
            s1h = spool.tile([128, HW], BF16, tag="c2a", name="c2a", bufs=2)
            nc.vector.tensor_scalar(out=s1h[:], in0=zi, scalar1=a1_2[:, 0:1],
                                    scalar2=0.5, op0=OP.is_gt, op1=OP.subtract)
            s2h = spool.tile([128, HW], BF16, tag="c2b", name="c2b", bufs=2)
            nc.gpsimd.tensor_scalar(out=s2h[:], in0=zi, scalar1=a2_2[:, 0:1],
                                    scalar2=0.5, op0=OP.is_ge, op1=OP.subtract)
            # t into slab A (fp8, exact {-1,0,1})
            nc.vector.tensor_tensor(
                out=pt[:, 1, 1:29, 2:30],
                in0=s1h[:].rearrange("p (a b) -> p a b", a=H),
                in1=s2h[:].rearrange("p (a b) -> p a b", a=H), op=OP.add)
            # shifted copies: B (ACT), C (DVE)
            nc.scalar.activation(out=pt[:, 0, 1:29, 1:29],
                                 in_=pt[:, 1, 1:29, 2:30], func=AF.Copy)
            nc.vector.tensor_copy(pt[:, 2, 0:28, 2:30], pt[:, 1, 1:29, 2:30])

        P4_PAIRS = [  # (slab_lo, dy, dx, w-pair index) ; rhs = slabs lo,lo+1
            (0, 0, 0, 0),   # taps (1, 0)
            (0, 1, 0, 1),   # taps (4, 3)
            (0, 2, 0, 2),   # taps (7, 6)
            (1, 0, 2, 3),   # taps (2, 5)
            (1, 2, 2, 4),   # taps (8, zero)
        ]

        def p4_conv(img):
            pt = pads[img % 3]
            zp = psum.tile([128, 2, 512], F32, tag="zp", name="zp", bufs=4)
            for hh in range(2):
                for i, (lo, dy, dx, k) in enumerate(P4_PAIRS):
                    rhs = pt[:, lo:lo + 2, dy + 14 * hh:dy + 14 * hh + 14,
                             dx + 1:dx + 29]
                    nc.tensor.matmul(zp[:, hh, 0:392],
                                     w2s[:, 2 * k:2 * k + 2, :], rhs,
                                     start=(i == 0), stop=(i == 4),
                                     perf_mode=PM.DoubleRow)
            nc.scalar.activation(
                out=z2[:, img, :].rearrange("p (h s) -> p h s", h=2),
                in_=zp[:, :, 0:392], func=AF.Copy,
                accum_out=stz[(3, "x")][:, img:img + 1])
            if img % 2 == 1:
                pr = img // 2
                zs = z2[:, img - 1:img + 1, :]
                d2t = spool.tile([128, 2, HW], BF16, tag="zsq", name="zsq",
                                 bufs=1)
                nc.vector.scalar_tensor_tensor(
                    out=d2t[:], in0=zs, scalar=1.0, in1=zs,
                    op0=OP.mult, op1=OP.mult,
                    accum_out=stz[(3, "q")][:, pr:pr + 1])
                dat = spool.tile([128, 2, HW], BF16, tag="zab", name="zab",
                                 bufs=1)
                nc.scalar.activation(
                    out=dat[:], in_=zs, func=AF.Abs,
                    accum_out=stz[(3, "a")][:, pr:pr + 1])

        for img in range(IMGS + 1):
            if img < IMGS:
                p4_cmp(img)
            if img >= 1:
                p4_conv(img - 1)

        pk3 = stpool.tile([128, 3], F32, tag="pk3", name="pk3")
        for i, k in enumerate(("x", "q", "a")):
            nc.vector.tensor_reduce(out=pk3[:, i:i + 1], in_=stz[(3, k)][:],
                                    axis=mybir.AxisListType.X, op=OP.add)
        ar3i = dram.tile([128, 3], F32, tag="ar3i", name="ar3i")
        ar3o = dram.tile([128, 3], F32, tag="ar3o", name="ar3o",
                         addr_space="Shared")
        nc.sync.dma_start(out=ar3i[:], in_=pk3[:])
        allreduce([ar3i.opt()], [ar3o.opt()])
        gp3 = stpool.tile([128, 3], F32, tag="gp3", name="gp3")
        nc.sync.dma_start(out=gp3[:], in_=ar3o[:])

        st3 = _stats_stage1(nc, tiny, "th3", 1, gp3[:, 0:1], gp3[:, 1:2],
                            g3c, al2, N1, ginv=g3i, alphainv=al2i)
        a1_3, a2_3, na1_3, na2_3 = _stats_stage2(nc, tiny, "th3", st3,
                                                 gp3[:, 2:3], b3c, NTOT2)

        # ============ P5: ternarize L3, conv3 (DR), residual, store ============
        p5_tiles = {}

        def p5_cmp(img):
            eng = L3_ENG[img]
            if eng == 'v':
                s3 = spool.tile([128, 2, HW], FP8, tag="s3v", name="s3v",
                                bufs=2)
                nc.vector.tensor_scalar(out=s3[:, 0, :], in0=z2[:, img, :],
                                        scalar1=a1_3[:, 0:1], scalar2=0.5,
                                        op0=OP.is_gt, op1=OP.subtract)
                nc.vector.tensor_scalar(out=s3[:, 1, :], in0=z2[:, img, :],
                                        scalar1=a2_3[:, 0:1], scalar2=0.5,
                                        op0=OP.is_ge, op1=OP.subtract)
            else:
                s3 = spool.tile([128, 2, HW], FP8, tag="s3a", name="s3a",
                                bufs=2)
                nc.scalar.activation(out=s3[:, 0, :], in_=z2[:, img, :],
                                     func=AF.Sign, bias=na1_3[:, 0:1],
                                     scale=1.0)
                nc.scalar.activation(out=s3[:, 1, :], in_=z2[:, img, :],
                                     func=AF.Sign, bias=na2_3[:, 0:1],
                                     scale=1.0)
            p5_tiles[img] = s3

        def p5_conv(img):
            s3 = p5_tiles.pop(img)
            alsc = al3h if L3_ENG[img] == 'a' else al3
            for qp in range(2):
                osb = opool.tile([128, 2, HW], F32, tag="osb", name="osb",
                                 bufs=2)
                for qi in range(2):
                    q = qp * 2 + qi
                    zp = psum.tile([128, 2, 512], F32, tag="zp", name="zp",
                                   bufs=4)
                    for hh in range(2):
                        nc.tensor.matmul(zp[:, hh, 0:392],
                                         w3s[:, 2 * q:2 * q + 2, :],
                                         s3[:, :, hh * 392:(hh + 1) * 392],
                                         start=True, stop=True,
                                         perf_mode=PM.DoubleRow)
                    nc.vector.scalar_tensor_tensor(
                        out=osb[:, qi, :].rearrange("p (h s) -> p h s", h=2),
                        in0=zp[:, :, 0:392], scalar=alsc[:, q:q + 1],
                        in1=xt[:, q, img, :].rearrange("p (h s) -> p h s",
                                                       h=2),
                        op0=OP.mult, op1=OP.add)
                st_eng = nc.sync if (img + qp) % 2 == 0 else nc.gpsimd
                st_eng.dma_start(out=out_d[img, qp], in_=osb[:])

        for img in range(IMGS + 1):
            if img < IMGS:
                p5_cmp(img)
            if img >= 1:
                p5_conv(img - 1)


def _build_nc(single_core=False, repeats=1, no_collective=False):
    nc = bacc.Bacc("TRN2", target_bir_lowering=False, debug=False,
                   num_devices=1 if single_core else N_CORES)
    x_d = nc.dram_tensor("x", [IMGS, 4, 128, HW], F32, kind="ExternalInput")
    w1_d = nc.dram_tensor("w1t", [8, 128, 128], FP8, kind="ExternalInput")
    w2_d = nc.dram_tensor("w2t", [10, 128, 128], FP8, kind="ExternalInput")
    w3_d = nc.dram_tensor("w3t", [8, 128, 128], FP8, kind="ExternalInput")
    cst_d = nc.dram_tensor("cst", [128, 30], F32, kind="ExternalInput")
    out_d = nc.dram_tensor("out", [IMGS, 2, 128, 2, HW], F32,
                           kind="ExternalOutput")
    with tile.TileContext(nc) as tc, ExitStack() as ctx:
        _emit(ctx, tc, x_d.ap(), w1_d.ap(), w2_d.ap(), w3_d.ap(),
              cst_d.ap(), out_d.ap(), single_core=single_core,
              repeats=repeats, no_collective=no_collective)
    nc.compile()
    return nc


def get_nc():
    if "nc" not in _CACHE:
        _CACHE["nc"] = _build_nc()
    return _CACHE["nc"]


# ----------------------------------------------------------------------------
# host-side wrapper
# ----------------------------------------------------------------------------

def prep_inputs(x, g1, b1, w1, g2, b2, w2, g3, b3, w3):
    """Host-side marshalling: shard x, binarize weights, pack constants."""
    x = np.asarray(x, np.float32)
    g1 = np.asarray(g1, np.float32); b1 = np.asarray(b1, np.float32)
    g2 = np.asarray(g2, np.float32); b2 = np.asarray(b2, np.float32)
    g3 = np.asarray(g3, np.float32); b3 = np.asarray(b3, np.float32)
    w1 = np.asarray(w1, np.float32); w2 = np.asarray(w2, np.float32)
    w3 = np.asarray(w3, np.float32)

    xs = x.reshape(N_CORES, IMGS, 4, 128, HW)

    FP8NP = ml_dtypes.float8_e4m3

    sg1 = np.sign(w1[:, :, 0, 0])                       # [co=128, ci=512]
    al1 = np.abs(w1).mean(axis=(1, 2, 3))               # [128]
    w1q = sg1.T.reshape(4, 128, 128)                    # [q, ci, co]
    w1t = np.ascontiguousarray(np.repeat(w1q, 2, axis=0)).astype(FP8NP)

    sg2 = np.sign(w2)                                   # [co,ci,3,3]
    al2 = np.abs(w2).mean(axis=(1, 2, 3))
    w2tap = sg2.transpose(2, 3, 1, 0).reshape(9, 128, 128)   # [tap, ci, co]
    # DR pair order: [w1,w0, w4,w3, w7,w6, w2,w5, w8,0]
    w2p = np.zeros((10, 128, 128), np.float32)
    order = [1, 0, 4, 3, 7, 6, 2, 5, 8]
    for i, t in enumerate(order):
        w2p[i] = w2tap[t]
    w2t = np.ascontiguousarray(w2p).astype(FP8NP)

    sg3 = np.sign(w3[:, :, 0, 0])                       # [co=512, ci=128]
    al3 = np.abs(w3).mean(axis=(1, 2, 3))               # [512]
    w3q = sg3.reshape(4, 128, 128).transpose(0, 2, 1)   # [q, ci, co]
    w3t = np.ascontiguousarray(np.repeat(w3q, 2, axis=0)).astype(FP8NP)

    cst = np.zeros((128, 30), np.float32)
    cst[:, 0:4] = g1.reshape(4, 128).T
    cst[:, 4:8] = b1.reshape(4, 128).T
    cst[:, 8] = al1
    cst[:, 9] = g2
    cst[:, 10] = b2
    cst[:, 11] = al2
    cst[:, 12] = g3
    cst[:, 13] = b3
    cst[:, 14:18] = al3.reshape(4, 128).T
    cst[:, 18:22] = (np.float32(1.0) / g1).reshape(4, 128).T
    cst[:, 22] = np.float32(1.0) / al1
    cst[:, 23] = np.float32(1.0) / g2
    cst[:, 24] = np.float32(1.0) / al2
    cst[:, 25] = np.float32(1.0) / g3
    cst[:, 26:30] = (np.float32(0.5) * al3).reshape(4, 128).T

    in_maps = []
    for c in range(N_CORES):
        in_maps.append({
            "x": np.ascontiguousarray(xs[c]),
            "w1t": w1t, "w2t": w2t, "w3t": w3t, "cst": cst,
        })
    return in_maps


def assemble_output(results):
    # results[c]["out"]: [8, 2, 128, 2, 784] -> [64, 512, 28, 28]
    parts = [np.asarray(results[c]["out"]) for c in range(N_CORES)]
    y = np.stack(parts, axis=0)                 # [8, 8, 2, 128, 2, 784]
    # [c, img, qp, p, qi, hw] -> [c, img, qp, qi, p, hw]
    y = y.transpose(0, 1, 2, 4, 3, 5)
    return np.ascontiguousarray(
        y.reshape(64, 512, H, H)).astype(np.float32)


def kernel(x, g1, b1, w1, g2, b2, w2, g3, b3, w3, _trace=False):
    in_maps = prep_inputs(x, g1, b1, w1, g2, b2, w2, g3, b3, w3)
    nc = get_nc()
    res = run_bass_kernel_spmd(nc, in_maps, list(range(N_CORES)),
                               trace=_trace)
    _CACHE["last_result"] = res
    return assemble_output(res.results)


if __name__ == "__main__":
    # smoke build
    nc = get_nc()
    print("built ok:", nc)


# revision 49
# speedup vs baseline: 2.2368x; 2.2368x over previous
"""Trainium2 Bass kernel for nn_Bottleneck (TBN-style quantized bottleneck).

Reference computation (per reference.py):
    identity = x
    h = qconv(BN(x,g1,b1),  w1b, 1x1)          # ternary acts, binary weights
    h = qconv(BN(h,g2,b2),  w2b, 3x3 pad 1)
    h = qconv(BN(h,g3,b3),  w3b, 1x1)
    out = identity + h
where BN uses batch statistics over (N,H,W) (sync-BN across the batch),
ternarize(x) = (x>d) - (x<-d) with d = 0.7*mean|x| (global), and
binarize(w) = sign(w)*mean|w|_per_out_channel.

Sharding: data-parallel over batch, 8 images per core on 8 cores; BN stats
and ternary thresholds are synchronized with one small AllReduce per stats
barrier (4 total: layer-1 needs two, exact Sum|x-m|).

v3 design notes.  In this runtime the dominant execution cost is
per-instruction processing (~0.5 us/instruction), so the kernel is shaped
to MINIMIZE INSTRUCTION COUNT first and engine balance second:
  * All convs are fp8 DoubleRow matmuls.  conv1/conv3 stack the two
    ternary compare-parts (s-0.5 in {-0.5,+0.5}) as the two DR k-tiles
    against duplicated weights: W.(s1-0.5)+W.(s2-0.5) = W.t with no
    explicit ternarize add and no extra PE cost.  DVE images emit fp8
    half-form compares; ACT images use Sign (+-1, evac scale 0.5).
  * conv2 materializes t = s1h+s2h in {-1,0,1} fp8 into a 3-slab padded
    tile [B|A|C] where B is t shifted one column left and C one row up.
    A DR matmul over slabs (B,A) with weights [w_{t+1}, w_t] computes two
    horizontal taps at once; slabs (A,C) pair taps (2,5); tap 8 rides a
    DR with a zero weight slab.  5 matmuls per half-image instead of 18.
  * One activation-table set (Copy/Square/Abs/Sign/Abs_reciprocal_sqrt),
    rsqrt = table seed + 1 mult-only Newton step; 1/A = u*r*(1/g)*(1/a).
  * Outputs are stored per (img, q-pair) with a DRAM layout that matches
    SBUF ([IMGS, 2, 128, 2, HW]), halving store DMAs; loads are 2-image;
    big DMAs alternate the sync (HWDGE) and gpsimd (SWDGE) queues.
  * BN + next-layer ternarize folded into per-channel thresholds a1, a2 on
    raw integer conv outputs (z kept bf16-exact).
"""

import os
from contextlib import ExitStack

import numpy as np
import ml_dtypes

import concourse.bass as bass
import concourse.bacc as bacc
import concourse.tile as tile
import concourse.mybir as mybir
from concourse import bass_isa
from concourse.bass_utils import run_bass_kernel_spmd

F32 = mybir.dt.float32
BF16 = mybir.dt.bfloat16
FP8 = mybir.dt.float8e4
AF = mybir.ActivationFunctionType
OP = mybir.AluOpType
PM = mybir.MatmulPerfMode

N_CORES = 8
IMGS = 8          # images per core
HW = 784          # 28*28
H = 28
EPS = 1e-5
N1 = 64 * HW              # BN count per channel (global batch)
NTOT1 = 64 * 512 * HW     # element count for delta1
NTOT2 = 64 * 128 * HW     # element count for delta2/delta3

_CACHE = {}

# per-image engine for layer-1 ternarize compares:
#   'v' = DVE fp8 half-form, 'a' = ACT Sign fp8 (evac scale 0.5)
L1_ENG = ['v', 'a', 'v', 'a', 'v', 'a', 'v', 'a']
L3_ENG = ['v', 'a', 'v', 'a', 'v', 'a', 'v', 'a']


def _rsqrt(nc, pool, u, shape, tag):
    """r = 1/sqrt(u), u > 0: ACT table seed (~4e-5 rel) + 1 mult-only
    Newton step on DVE (-> ~3e-9, ~ulp)."""
    r = pool.tile(shape, F32, tag=f"{tag}_r0", name=f"{tag}_r0")
    nc.scalar.activation(out=r[:], in_=u[:], func=AF.Abs_reciprocal_sqrt)
    for i in range(1):
        w1 = pool.tile(shape, F32, tag=f"{tag}_w1_{i}", name=f"{tag}_w1_{i}")
        nc.vector.tensor_mul(w1[:], u[:], r[:])
        w2 = pool.tile(shape, F32, tag=f"{tag}_w2_{i}", name=f"{tag}_w2_{i}")
        nc.vector.tensor_mul(w2[:], w1[:], r[:])
        h = pool.tile(shape, F32, tag=f"{tag}_h_{i}", name=f"{tag}_h_{i}")
        nc.vector.tensor_scalar(out=h[:], in0=w2[:], scalar1=-0.5, scalar2=1.5,
                                op0=OP.mult, op1=OP.add)
        r2 = pool.tile(shape, F32, tag=f"{tag}_r_{i}", name=f"{tag}_r_{i}")
        nc.vector.tensor_mul(r2[:], r[:], h[:])
        r = r2
    return r


def _stats_stage1(nc, pool, tag, nchunk, sx, sq, gv, alpha, n_cnt,
                  ginv, alphainv=None):
    """Mean / rstd / slope from AllReduced Sum z, Sum z^2."""
    shape = [128, nchunk]

    def t(name):
        return pool.tile(shape, F32, tag=f"{tag}_{name}", name=f"{tag}_{name}")

    m = t("m")
    nc.vector.tensor_scalar(out=m[:], in0=sx[:], scalar1=1.0 / n_cnt,
                            scalar2=None, op0=OP.mult)
    negm = t("negm")
    nc.vector.tensor_scalar(out=negm[:], in0=m[:], scalar1=-1.0, scalar2=None,
                            op0=OP.mult)
    ex2 = t("ex2")
    nc.vector.tensor_scalar(out=ex2[:], in0=sq[:], scalar1=1.0 / n_cnt,
                            scalar2=None, op0=OP.mult)
    m2 = t("m2")
    nc.vector.tensor_mul(m2[:], m[:], m[:])
    v = t("v")
    nc.vector.tensor_sub(v[:], ex2[:], m2[:])
    if alpha is not None:
        asq = t("asq")
        nc.vector.tensor_mul(asq[:], alpha[:], alpha[:])
        vh = t("vh")
        nc.vector.tensor_mul(vh[:], v[:], asq[:])
    else:
        vh = v
    u = t("u")
    nc.vector.tensor_scalar(out=u[:], in0=vh[:], scalar1=EPS, scalar2=None,
                            op0=OP.add)
    r = _rsqrt(nc, pool, u, shape, f"{tag}_rs")
    A = t("A")
    nc.vector.tensor_mul(A[:], r[:], gv[:])
    if alpha is not None:
        A2 = t("A2")
        nc.vector.tensor_mul(A2[:], A[:], alpha[:])
        A = A2
    # 1/A = sqrt(u) * (1/g) * (1/alpha); sqrt(u) = u * r
    sq_u = t("squ")
    nc.vector.tensor_mul(sq_u[:], u[:], r[:])
    Ainv = t("Ainv")
    nc.vector.tensor_mul(Ainv[:], sq_u[:], ginv[:])
    if alphainv is not None:
        A3 = t("Ainv2")
        nc.vector.tensor_mul(A3[:], Ainv[:], alphainv[:])
        Ainv = A3
    return {"m": m, "negm": negm, "A": A, "Ainv": Ainv, "shape": shape}


def _stats_stage2(nc, pool, tag, st, sa, bv, n_tot, want_neg=True):
    """Thresholds from stage-1 stats + AllReduced Sum|z - m| (or Sum|z|)."""
    shape = st["shape"]
    m, A, Ainv = st["m"], st["A"], st["Ainv"]
    nchunk = shape[1]

    def t(name):
        return pool.tile(shape, F32, tag=f"{tag}_{name}", name=f"{tag}_{name}")

    say = t("say")
    nc.vector.tensor_mul(say[:], A[:], sa[:])
    srow = pool.tile([128, 1], F32, tag=f"{tag}_srow", name=f"{tag}_srow")
    if nchunk > 1:
        nc.vector.tensor_reduce(out=srow[:], in_=say[:],
                                axis=mybir.AxisListType.X, op=OP.add)
    else:
        nc.vector.tensor_copy(srow[:], say[:])
    sall = pool.tile([128, 1], F32, tag=f"{tag}_sall", name=f"{tag}_sall")
    nc.gpsimd.partition_all_reduce(sall[:], srow[:], 128, bass_isa.ReduceOp.add)
    delta = pool.tile([128, 1], F32, tag=f"{tag}_delta", name=f"{tag}_delta")
    nc.vector.tensor_scalar(out=delta[:], in0=sall[:], scalar1=0.7 / n_tot,
                            scalar2=None, op0=OP.mult)
    d1 = t("d1")
    nc.vector.tensor_scalar(out=d1[:], in0=bv[:], scalar1=delta[:], scalar2=-1.0,
                            op0=OP.subtract, op1=OP.mult)
    e1 = t("e1")
    nc.vector.tensor_mul(e1[:], d1[:], Ainv[:])
    a1 = t("a1")
    nc.vector.tensor_add(a1[:], e1[:], m[:])
    d2 = t("d2")
    nc.vector.tensor_scalar(out=d2[:], in0=bv[:], scalar1=delta[:], scalar2=-1.0,
                            op0=OP.add, op1=OP.mult)
    e2 = t("e2")
    nc.vector.tensor_mul(e2[:], d2[:], Ainv[:])
    a2 = t("a2")
    nc.vector.tensor_add(a2[:], e2[:], m[:])
    if not want_neg:
        return a1, a2, None, None
    na1 = t("na1")
    nc.vector.tensor_scalar(out=na1[:], in0=a1[:], scalar1=-1.0, scalar2=None,
                            op0=OP.mult)
    na2 = t("na2")
    nc.vector.tensor_scalar(out=na2[:], in0=a2[:], scalar1=-1.0, scalar2=None,
                            op0=OP.mult)
    return a1, a2, na1, na2


def _emit(ctx: ExitStack, tc: tile.TileContext, x_d, w1_d, w2_d, w3_d,
          cst_d, out_d, single_core=False, repeats=1, no_collective=False):
    nc = tc.nc

    def allreduce(ins, outs):
        if single_core or no_collective:
            nc.gpsimd.dma_start(out=outs[0], in_=ins[0])
        else:
            nc.gpsimd.collective_compute(
                "AllReduce", OP.add, replica_groups=[list(range(N_CORES))],
                ins=ins, outs=outs)

    xpool = ctx.enter_context(tc.tile_pool(name="xres", bufs=1))
    zpool = ctx.enter_context(tc.tile_pool(name="zres", bufs=1))
    wpool = ctx.enter_context(tc.tile_pool(name="wts", bufs=1))
    stpool = ctx.enter_context(tc.tile_pool(name="stats", bufs=1))
    tiny = ctx.enter_context(tc.tile_pool(name="tiny", bufs=1))
    spool = ctx.enter_context(tc.tile_pool(name="scratch", bufs=2))
    padp = ctx.enter_context(tc.tile_pool(name="pads", bufs=1))
    opool = ctx.enter_context(tc.tile_pool(name="outbuf", bufs=2))
    psum = ctx.enter_context(tc.tile_pool(name="psum", bufs=4, space="PSUM"))
    dram = ctx.enter_context(tc.tile_pool(name="dram", bufs=1, space="DRAM"))

    # ---- resident tensors ----
    xt = xpool.tile([128, 4, IMGS, HW], F32, tag="x", name="x")
    z1 = zpool.tile([128, IMGS, HW], BF16, tag="z1", name="z1")
    z2 = zpool.tile([128, IMGS, HW], BF16, tag="z2", name="z2")
    w1s = wpool.tile([128, 8, 128], FP8, tag="w1", name="w1")     # q dup
    w2s = wpool.tile([128, 10, 128], FP8, tag="w2", name="w2")    # tap pairs
    w3s = wpool.tile([128, 8, 128], FP8, tag="w3", name="w3")     # q dup
    csts = wpool.tile([128, 30], F32, tag="cst", name="cst")

    nc.sync.dma_start(out=w1s[:], in_=w1_d[:].rearrange("q k m -> k q m"))
    nc.sync.dma_start(out=w2s[:], in_=w2_d[:].rearrange("q k m -> k q m"))
    nc.sync.dma_start(out=w3s[:], in_=w3_d[:].rearrange("q k m -> k q m"))
    nc.sync.dma_start(out=csts[:], in_=cst_d[:])
    g1c = csts[:, 0:4]
    b1c = csts[:, 4:8]
    al1 = csts[:, 8:9]
    g2c = csts[:, 9:10]
    b2c = csts[:, 10:11]
    al2 = csts[:, 11:12]
    g3c = csts[:, 12:13]
    b3c = csts[:, 13:14]
    al3 = csts[:, 14:18]
    g1i = csts[:, 18:22]
    al1i = csts[:, 22:23]
    g2i = csts[:, 23:24]
    al2i = csts[:, 24:25]
    g3i = csts[:, 25:26]
    al3h = csts[:, 26:30]      # 0.5 * al3 (Sign-form conv3 images)

    # ---- stats accumulators ----
    st1x = stpool.tile([128, 8], F32, tag="st1x", name="st1x")    # sum x
    st1q = stpool.tile([128, 8], F32, tag="st1q", name="st1q")    # sum x^2
    st1a = stpool.tile([128, 32], F32, tag="st1a", name="st1a")   # sum |x-m|
    stz = {}
    for L in (2, 3):
        for k in ("x", "q", "a"):
            n = IMGS if k == "x" else 2
            stz[(L, k)] = stpool.tile([128, n], F32, tag=f"st{L}{k}",
                                      name=f"st{L}{k}")

    # conv2 padded tiles [B|A|C]: 3 rotating, fully zeroed once per rep
    pads = [padp.tile([128, 3, 30, 32], FP8, tag=f"pad{i}", name=f"pad{i}")
            for i in range(3)]

    for _rep in range(repeats):
        for p in pads:
            nc.gpsimd.memset(p[:], 0.0)
        nc.gpsimd.memset(st1a[:], 0.0)

        # ================= P1: load x + layer-1 stats =================
        # 2-image loads alternating HWDGE/SWDGE; per half-batch (4 imgs):
        # Sum x (DVE tensor_reduce) and Sum x^2 (ACT Square accum) per q.
        for pr in range(4):
            for sub in range(2):
                img = 2 * pr + sub
                ld_eng = nc.sync if img % 2 == 0 else nc.gpsimd
                ld_eng.dma_start(out=xt[:, :, img, :],
                                 in_=x_d[img].rearrange("q p s -> p q s"))
            if pr % 2 == 1:
                half = pr // 2
                for q in range(4):
                    xs = xt[:, q, 4 * half:4 * half + 4, :]
                    nc.vector.tensor_reduce(
                        out=st1x[:, q * 2 + half:q * 2 + half + 1],
                        in_=xs.rearrange("p a b -> p (a b)"),
                        axis=mybir.AxisListType.X, op=OP.add)
                    dw = spool.tile([128, 4, HW], BF16, tag="sqdump",
                                    name="sqdump", bufs=1)
                    nc.scalar.activation(
                        out=dw[:], in_=xs, func=AF.Square,
                        accum_out=st1q[:, q * 2 + half:q * 2 + half + 1])

        pk1 = stpool.tile([128, 8], F32, tag="pk1", name="pk1")
        nc.vector.tensor_reduce(
            out=pk1[:, 0:4], in_=st1x[:].rearrange("p (q h) -> p q h", q=4),
            axis=mybir.AxisListType.X, op=OP.add)
        nc.vector.tensor_reduce(
            out=pk1[:, 4:8], in_=st1q[:].rearrange("p (q h) -> p q h", q=4),
            axis=mybir.AxisListType.X, op=OP.add)
        ar1i = dram.tile([128, 8], F32, tag="ar1i", name="ar1i")
        ar1o = dram.tile([128, 8], F32, tag="ar1o", name="ar1o",
                         addr_space="Shared")
        nc.sync.dma_start(out=ar1i[:], in_=pk1[:])
        allreduce([ar1i.opt()], [ar1o.opt()])
        gp1 = stpool.tile([128, 8], F32, tag="gp1", name="gp1")
        nc.sync.dma_start(out=gp1[:], in_=ar1o[:])

        st1 = _stats_stage1(nc, tiny, "th1", 4, gp1[:, 0:4], gp1[:, 4:8],
                            g1c, None, N1, ginv=g1i)

        # ============ P2: exact Sum|x - m| pass ============
        # ACT 1-pass (Abs, bias=-m) imgs 0-4 (4-group + single); DVE 3-img
        # sub + abs-reduce for imgs 5-7.  st1a is pre-zeroed, so one
        # [128,4,8] reduce packs all partials.
        for q in range(4):
            dw3 = spool.tile([128, 4, HW], BF16, tag="sqdump",
                             name="absdump", bufs=1)
            nc.scalar.activation(out=dw3[:], in_=xt[:, q, 0:4, :],
                                 func=AF.Abs, bias=st1["negm"][:, q:q + 1],
                                 scale=1.0,
                                 accum_out=st1a[:, q * 8:q * 8 + 1])
            dw1 = spool.tile([128, HW], BF16, tag="d784", name="absdump1",
                             bufs=2)
            nc.scalar.activation(out=dw1[:], in_=xt[:, q, 4, :],
                                 func=AF.Abs, bias=st1["negm"][:, q:q + 1],
                                 scale=1.0,
                                 accum_out=st1a[:, q * 8 + 1:q * 8 + 2])
            dfp = spool.tile([128, 3, HW], F32, tag="dfp3", name="dfp3", bufs=1)
            nc.vector.tensor_scalar(out=dfp[:], in0=xt[:, q, 5:8, :],
                                    scalar1=st1["m"][:, q:q + 1], scalar2=None,
                                    op0=OP.subtract)
            nc.vector.tensor_reduce(
                out=st1a[:, q * 8 + 2:q * 8 + 3],
                in_=dfp[:].rearrange("p a b -> p (a b)"),
                axis=mybir.AxisListType.X, op=OP.add,
                apply_absolute_value=True)
        pka = stpool.tile([128, 4], F32, tag="pka", name="pka")
        nc.vector.tensor_reduce(
            out=pka[:], in_=st1a[:].rearrange("p (q c) -> p q c", q=4),
            axis=mybir.AxisListType.X, op=OP.add)
        arai = dram.tile([128, 4], F32, tag="arai", name="arai")
        arao = dram.tile([128, 4], F32, tag="arao", name="arao",
                         addr_space="Shared")
        nc.sync.dma_start(out=arai[:], in_=pka[:])
        allreduce([arai.opt()], [arao.opt()])
        gpa = stpool.tile([128, 4], F32, tag="gpa", name="gpa")
        nc.sync.dma_start(out=gpa[:], in_=arao[:])

        a1_1, a2_1, na1_1, na2_1 = _stats_stage2(nc, tiny, "th1", st1, gpa[:],
                                                 b1c, NTOT1)

        # ============ P3: ternarize L1 + conv1 (fp8 DR) + L2 stats ============
        p3_tiles = {}

        def p3_cmp(img):
            eng = L1_ENG[img]
            if eng == 'v':
                s8 = spool.tile([128, 4, 2, HW], FP8, tag="s8v", name="s8v",
                                bufs=1)
                for q in range(4):
                    nc.vector.tensor_scalar(
                        out=s8[:, q, 0, :], in0=xt[:, q, img, :],
                        scalar1=a1_1[:, q:q + 1], scalar2=0.5,
                        op0=OP.is_gt, op1=OP.subtract)
                    nc.vector.tensor_scalar(
                        out=s8[:, q, 1, :], in0=xt[:, q, img, :],
                        scalar1=a2_1[:, q:q + 1], scalar2=0.5,
                        op0=OP.is_ge, op1=OP.subtract)
            else:
                s8 = spool.tile([128, 4, 2, HW], FP8, tag="s8a", name="s8a",
                                bufs=1)
                for q in range(4):
                    nc.scalar.activation(out=s8[:, q, 0, :],
                                         in_=xt[:, q, img, :], func=AF.Sign,
                                         bias=na1_1[:, q:q + 1], scale=1.0)
                    nc.scalar.activation(out=s8[:, q, 1, :],
                                         in_=xt[:, q, img, :], func=AF.Sign,
                                         bias=na2_1[:, q:q + 1], scale=1.0)
            p3_tiles[img] = s8

        def p3_conv(img):
            s8 = p3_tiles.pop(img)
            evac_scale = 0.5 if L1_ENG[img] == 'a' else 1.0
            zp = psum.tile([128, 2, 512], F32, tag="zp", name="zp", bufs=4)
            # q-major so consecutive matmuls share weights (ldweights dedup)
            for q in range(4):
                for hh in range(2):
                    nc.tensor.matmul(
                        zp[:, hh, 0:392],
                        w1s[:, 2 * q:2 * q + 2, :],
                        s8[:, q, :, hh * 392:(hh + 1) * 392],
                        start=(q == 0), stop=(q == 3),
                        perf_mode=PM.DoubleRow, skip_group_check=True)
            nc.scalar.activation(
                out=z1[:, img, :].rearrange("p (h s) -> p h s", h=2),
                in_=zp[:, :, 0:392], func=AF.Copy, scale=evac_scale,
                accum_out=stz[(2, "x")][:, img:img + 1])
            if img % 4 == 3:
                pr = img // 4
                zs = z1[:, img - 3:img + 1, :]
                d2t = spool.tile([128, 4, HW], BF16, tag="zsq4", name="zsq",
                                 bufs=1)
                nc.vector.scalar_tensor_tensor(
                    out=d2t[:], in0=zs, scalar=1.0, in1=zs,
                    op0=OP.mult, op1=OP.mult,
                    accum_out=stz[(2, "q")][:, pr:pr + 1])
                dat = spool.tile([128, 4, HW], BF16, tag="sqdump", name="zab",
                                 bufs=1)
                nc.scalar.activation(
                    out=dat[:], in_=zs, func=AF.Abs,
                    accum_out=stz[(2, "a")][:, pr:pr + 1])

        for img in range(IMGS + 1):
            if img < IMGS:
                p3_cmp(img)
            if img >= 1:
                p3_conv(img - 1)

        pk2 = stpool.tile([128, 3], F32, tag="pk2", name="pk2")
        for i, k in enumerate(("x", "q", "a")):
            nc.vector.tensor_reduce(out=pk2[:, i:i + 1], in_=stz[(2, k)][:],
                                    axis=mybir.AxisListType.X, op=OP.add)
        ar2i = dram.tile([128, 3], F32, tag="ar2i", name="ar2i")
        ar2o = dram.tile([128, 3], F32, tag="ar2o", name="ar2o",
                         addr_space="Shared")
        nc.sync.dma_start(out=ar2i[:], in_=pk2[:])
        allreduce([ar2i.opt()], [ar2o.opt()])
        gp2 = stpool.tile([128, 3], F32, tag="gp2", name="gp2")
        nc.sync.dma_start(out=gp2[:], in_=ar2o[:])

        st2 = _stats_stage1(nc, tiny, "th2", 1, gp2[:, 0:1], gp2[:, 1:2],
                            g2c, al1, N1, ginv=g2i, alphainv=al1i)
        a1_2, a2_2, _, _ = _stats_stage2(nc, tiny, "th2", st2, gp2[:, 2:3],
                                         b2c, NTOT2, want_neg=False)

        # ============ P4: ternarize L2 -> 3-slab padded fp8, conv2 ============
        # slabs [B|A|C]: A = t at interior [1:29, 2:30]; B = t at [1:29,
        # 1:29] (column -1); C = t at [0:28, 2:30] (row -1).  DR pairs:
        # (B,A) with weights [w_{t+1}, w_t] for t in {0,3,6}; (A,C) with
        # [w2, w5]; (A,C) with [w8, 0].
        def p4_cmp(img):
            pt = pads[img % 3]
            zi = z1[:, img, :]
            s1h = spool.tile([128, HW], BF16, tag="c2a", name="c2a", bufs=2)
            nc.vector.tensor_scalar(out=s1h[:], in0=zi, scalar1=a1_2[:, 0:1],
                                    scalar2=0.5, op0=OP.is_gt, op1=OP.subtract)
            s2h = spool.tile([128, HW], BF16, tag="c2b", name="c2b", bufs=2)
            nc.gpsimd.tensor_scalar(out=s2h[:], in0=zi, scalar1=a2_2[:, 0:1],
                                    scalar2=0.5, op0=OP.is_ge, op1=OP.subtract)
            # t into slab A (fp8, exact {-1,0,1})
            nc.vector.tensor_tensor(
                out=pt[:, 1, 1:29, 2:30],
                in0=s1h[:].rearrange("p (a b) -> p a b", a=H),
                in1=s2h[:].rearrange("p (a b) -> p a b", a=H), op=OP.add)
            # shifted copies: B (ACT), C (DVE)
            nc.scalar.activation(out=pt[:, 0, 1:29, 1:29],
                                 in_=pt[:, 1, 1:29, 2:30], func=AF.Copy)
            nc.vector.tensor_copy(pt[:, 2, 0:28, 2:30], pt[:, 1, 1:29, 2:30])

        P4_PAIRS = [  # (slab_lo, dy, dx, w-pair index) ; rhs = slabs lo,lo+1
            (0, 0, 0, 0),   # taps (1, 0)
            (0, 1, 0, 1),   # taps (4, 3)
            (0, 2, 0, 2),   # taps (7, 6)
            (1, 0, 2, 3),   # taps (2, 5)
            (1, 2, 2, 4),   # taps (8, zero)
        ]

        def p4_conv_wave(im0):
            # two images share each weight load (k-outer, img/hh-inner)
            zps = {}
            for img in (im0, im0 + 1):
                zps[img] = psum.tile([128, 2, 512], F32, tag="zp", name="zp",
                                     bufs=4)
            for i, (lo, dy, dx, k) in enumerate(P4_PAIRS):
                for img in (im0, im0 + 1):
                    pt = pads[img % 3]
                    rhs = pt[:, lo:lo + 2, dy:dy + 14, dx + 1:dx + 29]
                    nc.tensor.matmul(zps[img][:, 0, 0:392],
                                     w2s[:, 2 * k:2 * k + 2, :], rhs,
                                     start=(i == 0), stop=(i == 4),
                                     perf_mode=PM.DoubleRow,
                                     skip_group_check=True)
                    rhs = pt[:, lo:lo + 2, dy + 14:dy + 28, dx + 1:dx + 29]
                    nc.tensor.matmul(zps[img][:, 1, 0:392],
                                     w2s[:, 2 * k:2 * k + 2, :], rhs,
                                     start=(i == 0), stop=(i == 4),
                                     perf_mode=PM.DoubleRow,
                                     skip_group_check=True)
            for img in (im0, im0 + 1):
                nc.scalar.activation(
                    out=z2[:, img, :].rearrange("p (h s) -> p h s", h=2),
                    in_=zps[img][:, :, 0:392], func=AF.Copy,
                    accum_out=stz[(3, "x")][:, img:img + 1])
            if im0 % 4 == 2:
                pr = im0 // 4
                zs = z2[:, im0 - 2:im0 + 2, :]
                d2t = spool.tile([128, 4, HW], BF16, tag="zsq4", name="zsq",
                                 bufs=1)
                nc.vector.scalar_tensor_tensor(
                    out=d2t[:], in0=zs, scalar=1.0, in1=zs,
                    op0=OP.mult, op1=OP.mult,
                    accum_out=stz[(3, "q")][:, pr:pr + 1])
                dat = spool.tile([128, 4, HW], BF16, tag="sqdump", name="zab",
                                 bufs=1)
                nc.scalar.activation(
                    out=dat[:], in_=zs, func=AF.Abs,
                    accum_out=stz[(3, "a")][:, pr:pr + 1])

        # stagger: compares stay one image ahead of the 2-image conv waves
        p4_cmp(0)
        p4_cmp(1)
        for w in range(4):
            if 2 * w + 2 < IMGS:
                p4_cmp(2 * w + 2)
            p4_conv_wave(2 * w)
            if 2 * w + 3 < IMGS:
                p4_cmp(2 * w + 3)

        pk3 = stpool.tile([128, 3], F32, tag="pk3", name="pk3")
        for i, k in enumerate(("x", "q", "a")):
            nc.vector.tensor_reduce(out=pk3[:, i:i + 1], in_=stz[(3, k)][:],
                                    axis=mybir.AxisListType.X, op=OP.add)
        ar3i = dram.tile([128, 3], F32, tag="ar3i", name="ar3i")
        ar3o = dram.tile([128, 3], F32, tag="ar3o", name="ar3o",
                         addr_space="Shared")
        nc.sync.dma_start(out=ar3i[:], in_=pk3[:])
        allreduce([ar3i.opt()], [ar3o.opt()])
        gp3 = stpool.tile([128, 3], F32, tag="gp3", name="gp3")
        nc.sync.dma_start(out=gp3[:], in_=ar3o[:])

        st3 = _stats_stage1(nc, tiny, "th3", 1, gp3[:, 0:1], gp3[:, 1:2],
                            g3c, al2, N1, ginv=g3i, alphainv=al2i)
        a1_3, a2_3, na1_3, na2_3 = _stats_stage2(nc, tiny, "th3", st3,
                                                 gp3[:, 2:3], b3c, NTOT2)

        # ============ P5: ternarize L3, conv3 (DR), residual, store ============
        p5_tiles = {}

        def p5_cmp(img):
            eng = L3_ENG[img]
            if eng == 'v':
                s3 = spool.tile([128, 2, HW], FP8, tag="s3v", name="s3v",
                                bufs=2)
                nc.vector.tensor_scalar(out=s3[:, 0, :], in0=z2[:, img, :],
                                        scalar1=a1_3[:, 0:1], scalar2=0.5,
                                        op0=OP.is_gt, op1=OP.subtract)
                nc.vector.tensor_scalar(out=s3[:, 1, :], in0=z2[:, img, :],
                                        scalar1=a2_3[:, 0:1], scalar2=0.5,
                                        op0=OP.is_ge, op1=OP.subtract)
            else:
                s3 = spool.tile([128, 2, HW], FP8, tag="s3a", name="s3a",
                                bufs=2)
                nc.scalar.activation(out=s3[:, 0, :], in_=z2[:, img, :],
                                     func=AF.Sign, bias=na1_3[:, 0:1],
                                     scale=1.0)
                nc.scalar.activation(out=s3[:, 1, :], in_=z2[:, img, :],
                                     func=AF.Sign, bias=na2_3[:, 0:1],
                                     scale=1.0)
            p5_tiles[img] = s3

        def p5_conv_wave(im0):
            # two images (one DVE-form, one ACT-form) share weight loads
            imgs = (im0, im0 + 1)
            s3s = {img: p5_tiles.pop(img) for img in imgs}
            for qp in range(2):
                osbs = {img: opool.tile([128, 2, HW], F32, tag="osb",
                                        name="osb", bufs=2) for img in imgs}
                for qi in range(2):
                    q = qp * 2 + qi
                    zps = {}
                    for img in imgs:
                        zps[img] = psum.tile([128, 2, 512], F32, tag="zp",
                                             name="zp", bufs=4)
                        for hh in range(2):
                            nc.tensor.matmul(
                                zps[img][:, hh, 0:392],
                                w3s[:, 2 * q:2 * q + 2, :],
                                s3s[img][:, :, hh * 392:(hh + 1) * 392],
                                start=True, stop=True,
                                perf_mode=PM.DoubleRow, skip_group_check=True)
                    for img in imgs:
                        alsc = al3h if L3_ENG[img] == 'a' else al3
                        nc.vector.scalar_tensor_tensor(
                            out=osbs[img][:, qi, :].rearrange(
                                "p (h s) -> p h s", h=2),
                            in0=zps[img][:, :, 0:392], scalar=alsc[:, q:q + 1],
                            in1=xt[:, q, img, :].rearrange(
                                "p (h s) -> p h s", h=2),
                            op0=OP.mult, op1=OP.add)
                for img in imgs:
                    st_eng = nc.sync if (img + qp) % 2 == 0 else nc.gpsimd
                    st_eng.dma_start(out=out_d[img, qp], in_=osbs[img][:])

        p5_cmp(0)
        p5_cmp(1)
        for w in range(4):
            if 2 * w + 2 < IMGS:
                p5_cmp(2 * w + 2)
            p5_conv_wave(2 * w)
            if 2 * w + 3 < IMGS:
                p5_cmp(2 * w + 3)


def _dedup_ldweights(nc):
    """Remove InstLdweights that reload the identical weights already in
    the PE array (same AP/perf-mode, immediately consecutive in the PE
    stream).  Their waits are merged into the following instruction."""
    removed = 0
    for b in nc.m.functions[0].blocks:
        insts = b.instructions
        keep = []
        last_sig = None
        for i in insts:
            tn = type(i).__name__
            if tn == 'InstLdweights':
                sig = (str(i.ins[0]), str(i.perf_mode), str(i.is_transpose),
                       str(i.tile_position), str(i.tile_size))
                if sig == last_sig:
                    # merge waits into the next PE instruction
                    si = i.sync_info
                    if si is not None and len(si.on_wait) > 0:
                        _PENDING.extend(si.on_wait)
                    removed += 1
                    continue
                last_sig = sig
            elif tn == 'InstMatmult':
                if _PENDING:
                    si = i.sync_info
                    if si is None:
                        i.sync_info = mybir.SyncInfo(on_wait=list(_PENDING),
                                                     on_update=[])
                    else:
                        si.on_wait = list(si.on_wait) + list(_PENDING)
                    _PENDING.clear()
            else:
                # any other PE instruction invalidates the loaded weights
                if str(getattr(i, 'engine', '')) == 'EngineType.PE':
                    last_sig = None
            keep.append(i)
        del insts[:]
        insts.extend(keep)
    return removed


_PENDING = []


def _build_nc(single_core=False, repeats=1, no_collective=False):
    nc = bacc.Bacc("TRN2", target_bir_lowering=False, debug=False,
                   num_devices=1 if single_core else N_CORES)
    x_d = nc.dram_tensor("x", [IMGS, 4, 128, HW], F32, kind="ExternalInput")
    w1_d = nc.dram_tensor("w1t", [8, 128, 128], FP8, kind="ExternalInput")
    w2_d = nc.dram_tensor("w2t", [10, 128, 128], FP8, kind="ExternalInput")
    w3_d = nc.dram_tensor("w3t", [8, 128, 128], FP8, kind="ExternalInput")
    cst_d = nc.dram_tensor("cst", [128, 30], F32, kind="ExternalInput")
    out_d = nc.dram_tensor("out", [IMGS, 2, 128, 2, HW], F32,
                           kind="ExternalOutput")
    with tile.TileContext(nc) as tc, ExitStack() as ctx:
        _emit(ctx, tc, x_d.ap(), w1_d.ap(), w2_d.ap(), w3_d.ap(),
              cst_d.ap(), out_d.ap(), single_core=single_core,
              repeats=repeats, no_collective=no_collective)
    _dedup_ldweights(nc)
    nc.compile()
    return nc


def get_nc():
    if "nc" not in _CACHE:
        _CACHE["nc"] = _build_nc()
    return _CACHE["nc"]


# ----------------------------------------------------------------------------
# host-side wrapper
# ----------------------------------------------------------------------------

def prep_inputs(x, g1, b1, w1, g2, b2, w2, g3, b3, w3):
    """Host-side marshalling: shard x, binarize weights, pack constants."""
    x = np.asarray(x, np.float32)
    g1 = np.asarray(g1, np.float32); b1 = np.asarray(b1, np.float32)
    g2 = np.asarray(g2, np.float32); b2 = np.asarray(b2, np.float32)
    g3 = np.asarray(g3, np.float32); b3 = np.asarray(b3, np.float32)
    w1 = np.asarray(w1, np.float32); w2 = np.asarray(w2, np.float32)
    w3 = np.asarray(w3, np.float32)

    xs = x.reshape(N_CORES, IMGS, 4, 128, HW)

    FP8NP = ml_dtypes.float8_e4m3

    sg1 = np.sign(w1[:, :, 0, 0])                       # [co=128, ci=512]
    al1 = np.abs(w1).mean(axis=(1, 2, 3))               # [128]
    w1q = sg1.T.reshape(4, 128, 128)                    # [q, ci, co]
    w1t = np.ascontiguousarray(np.repeat(w1q, 2, axis=0)).astype(FP8NP)

    sg2 = np.sign(w2)                                   # [co,ci,3,3]
    al2 = np.abs(w2).mean(axis=(1, 2, 3))
    w2tap = sg2.transpose(2, 3, 1, 0).reshape(9, 128, 128)   # [tap, ci, co]
    # DR pair order: [w1,w0, w4,w3, w7,w6, w2,w5, w8,0]
    w2p = np.zeros((10, 128, 128), np.float32)
    order = [1, 0, 4, 3, 7, 6, 2, 5, 8]
    for i, t in enumerate(order):
        w2p[i] = w2tap[t]
    w2t = np.ascontiguousarray(w2p).astype(FP8NP)

    sg3 = np.sign(w3[:, :, 0, 0])                       # [co=512, ci=128]
    al3 = np.abs(w3).mean(axis=(1, 2, 3))               # [512]
    w3q = sg3.reshape(4, 128, 128).transpose(0, 2, 1)   # [q, ci, co]
    w3t = np.ascontiguousarray(np.repeat(w3q, 2, axis=0)).astype(FP8NP)

    cst = np.zeros((128, 30), np.float32)
    cst[:, 0:4] = g1.reshape(4, 128).T
    cst[:, 4:8] = b1.reshape(4, 128).T
    cst[:, 8] = al1
    cst[:, 9] = g2
    cst[:, 10] = b2
    cst[:, 11] = al2
    cst[:, 12] = g3
    cst[:, 13] = b3
    cst[:, 14:18] = al3.reshape(4, 128).T
    cst[:, 18:22] = (np.float32(1.0) / g1).reshape(4, 128).T
    cst[:, 22] = np.float32(1.0) / al1
    cst[:, 23] = np.float32(1.0) / g2
    cst[:, 24] = np.float32(1.0) / al2
    cst[:, 25] = np.float32(1.0) / g3
    cst[:, 26:30] = (np.float32(0.5) * al3).reshape(4, 128).T

    in_maps = []
    for c in range(N_CORES):
        in_maps.append({
            "x": np.ascontiguousarray(xs[c]),
            "w1t": w1t, "w2t": w2t, "w3t": w3t, "cst": cst,
        })
    return in_maps


def assemble_output(results):
    # results[c]["out"]: [8, 2, 128, 2, 784] -> [64, 512, 28, 28]
    parts = [np.asarray(results[c]["out"]) for c in range(N_CORES)]
    y = np.stack(parts, axis=0)                 # [8, 8, 2, 128, 2, 784]
    # [c, img, qp, p, qi, hw] -> [c, img, qp, qi, p, hw]
    y = y.transpose(0, 1, 2, 4, 3, 5)
    return np.ascontiguousarray(
        y.reshape(64, 512, H, H)).astype(np.float32)


def kernel(x, g1, b1, w1, g2, b2, w2, g3, b3, w3, _trace=False):
    in_maps = prep_inputs(x, g1, b1, w1, g2, b2, w2, g3, b3, w3)
    nc = get_nc()
    res = run_bass_kernel_spmd(nc, in_maps, list(range(N_CORES)),
                               trace=_trace)
    _CACHE["last_result"] = res
    return assemble_output(res.results)


if __name__ == "__main__":
    # smoke build
    nc = get_nc()
    print("built ok:", nc)
